# revision 20
# baseline (speedup 1.0000x reference)
"""Trainium2 Bass kernel for fused attention + LayerNorm + projection.

Computation (per reference):
    q = input1 @ Wq + bq                       [8192, 32]
    k = input2 @ Wk + bk                       [8192, 32]
    v = input2 @ Wv + bv                       [8192, 32]
    P = softmax(q @ k.T, axis=-1)              [8192, 8192]
    fused = P @ v                              [8192, 32]
    out = LayerNorm(fused) * gamma + beta @ Wo + bo   [8192, 128]

Sharding: data-parallel over rows of input1 (1024 rows per core, 8 cores);
input2 and weights replicated.

Algebraic simplifications (validated vs reference, rel err ~6e-3):
  - softmax normalization (and max-subtraction) skipped: LayerNorm is
    invariant to a positive per-row scale, so exp(s) @ v is normalized for
    free by LN.
  - gamma folded into Wo (diag(gamma) @ Wo), beta/bo folded into an extra
    contraction row via an augmented ones-row, on the host.

Dataflow per core (ACT-exp is the roofline: 8.4M exps at 1 elem/lane/cyc
@1.2GHz ~= 64us; everything else is arranged to overlap under it):
  - x2 is cast to bf16 on host and DMA'd HBM->SBUF through the XBAR
    transpose unit: x2T [128d, 8192n] lands directly with the contraction
    dim on partitions - zero PE transposes for the k/v projections.
  - prep is batched 4 groups (16 chunks) per dedicated 2-bank PSUM tile:
    4x 4-way column-tiled k quads into bank0 (kT banded into kstack f32r,
    chunk c at partitions 32*(c%4)) and 16 v matmuls into bank1 (v
    natural [n,32] -> vstack bf16), then ONE bias-add each. All 4 batches
    are emitted up front so prep races ahead of the attention stream
    during its PE-idle ramp.
  - q: x1 stays f32 for precision; PE-transposed (bank-alternated), then
    a 4-way col-tiled f32r matmul replicates qT into all 4 partition
    bands of qt_rep (fp32 matmuls would split into HI/LO pairs and run
    4x slower - everything on the PE is f32r or bf16).
  - scores: per (pass, group) ONE 4-way row-tiled f32r matmul quad
    (tile_position=(32j,0), concurrent on the PE sub-arrays) writes 4
    PSUM banks = scoresT for 4 chunks x 512 m.
  - exp on ACT straight out of PSUM, 2x [128,1024] bf16 -> pt. Emission
    is software-pipelined (scores g | exp g | AV g-1) so the PE finishes
    each score quad inside the previous exp's shadow and ACT never gaps.
  - AV: 4-way col-tiled bf16 quad accumulates fusedT into one PSUM bank
    across all 16 groups (start on g==0, stop on g==15).
  - m is processed in 2 sequential passes of 512 rows; pass 0's LayerNorm
    + output projection overlaps pass 1's attention stream.
  - PSUM: 2x [128,1024] score tiles (4 banks) + prep tile (2 banks) +
    1 AV bank + 1 LN bank = 8.
  - LayerNorm (batched, 4 blocks per pass): bn_stats/bn_aggr; rstd is
    computed on the DVE with the bit-trick rsqrt + 2 Newton steps so the
    ACT engine NEVER leaves the exp table set (a Ln/Sqrt activation would
    cost two ~2.7us ACT_TABLE_LOAD+DRAIN stalls per pass); projection
    through an augmented [33,512] lhsT with a persistent ones row; one
    rearranged output DMA per pass.
"""

import os
import sys

import numpy as np

N1 = 8192
N2 = 8192
DIN = 128
D = 32
DOUT = 128
NCORES = 8
MSH = N1 // NCORES          # rows per core
NCH = N2 // 128             # 64 in2 chunks
NG = NCH // 4               # 16 groups of 4 chunks
NB = NG // 4                # 4 prep batches of 4 groups
NP = MSH // 512             # 2 m-passes of 512 rows
LN_EPS = 1e-5

_CACHE = {}


def _import_concourse():
    try:
        import concourse.bass  # noqa: F401
    except ImportError:
        for p in ("/opt/trn_rl_repo", os.path.expanduser("~/.axon_site/_ro/trn_rl_repo")):
            if os.path.isdir(p) and p not in sys.path:
                sys.path.insert(0, p)


# Packed f32 consts layout (columns of the [128, CW] "cst" tensor).
C_ID = 0          # ident [128,128]
C_BV = 128        # bv4   [128,1]
C_BQ = 129        # bq4   [128,1]
C_BK = 130        # bk4   [128,1]
C_EPS = 131       # epsc  [128,1]
C_WOA = 132       # woa   [33,128] (rows 0:33)
C_WQ = 260        # wq4   [128,128] (Wq tiled 4x in cols)
C_BV16 = 388      # bv16  [128,512]
CW = 900


def build():
    """Build (and cache) the compiled single-core SPMD Bass program."""
    if "nc" in _CACHE:
        return _CACHE["nc"]
    _import_concourse()
    import concourse.bacc as bacc
    import concourse.tile as tile
    from concourse import mybir

    f32 = mybir.dt.float32
    f32r = mybir.dt.float32r
    i32 = mybir.dt.int32
    bf16 = mybir.dt.bfloat16
    AF = mybir.ActivationFunctionType
    OP = mybir.AluOpType

    nc = bacc.Bacc(None, target_bir_lowering=False, debug=False)

    x1 = nc.dram_tensor("x1", [MSH, DIN], f32, kind="ExternalInput")
    x2b = nc.dram_tensor("x2b", [N2, DIN], bf16, kind="ExternalInput")
    cst_d = nc.dram_tensor("cst", [128, CW], f32, kind="ExternalInput")
    id_d = nc.dram_tensor("identd", [128, 128], f32, kind="ExternalInput")
    wkv_d = nc.dram_tensor("wkv", [DIN, 2 * D], bf16, kind="ExternalInput")
    out_d = nc.dram_tensor("out", [MSH, DOUT], f32, kind="ExternalOutput")

    from contextlib import ExitStack

    with tile.TileContext(nc) as tc, ExitStack() as outer:
        consts = outer.enter_context(tc.tile_pool(name="consts", bufs=1))
        persist = outer.enter_context(tc.tile_pool(name="persist", bufs=1))

        cst = consts.tile([128, CW], f32)
        wkv = consts.tile([DIN, 2 * D], bf16)
        identt = consts.tile([128, 128], f32)
        ident = identt[:]
        bv4 = cst[:, C_BV:C_BV + 1]
        bv16 = cst[:, C_BV16:C_BV16 + 512]
        bq4 = cst[:, C_BQ:C_BQ + 1]
        bk4 = cst[:, C_BK:C_BK + 1]
        epsc = cst[:, C_EPS:C_EPS + 1]
        wk = wkv[:, 0:D]
        wv = wkv[:, D:2 * D]

        woa_r = consts.tile([D + 1, DOUT], f32r)
        nc.vector.tensor_copy(woa_r, cst[0:D + 1, C_WOA:C_WOA + 128])
        wq4_r = consts.tile([DIN, 128], f32r)
        nc.vector.tensor_copy(wq4_r, cst[:, C_WQ:C_WQ + 128])
        magic = consts.tile([128, NP * 4], i32)
        nc.vector.memset(magic, 0x5F3759DF)

        # Pull the exp table load (~2.7us) into the startup window; source
        # from a memset tile so it gates on no DMA (the ACT DGE queue
        # carries the second x1 half right behind it).
        warm = consts.tile([1, 8], f32)
        nc.vector.memset(warm, 0.0)
        nc.scalar.activation(warm, warm, AF.Exp)

        x2t = persist.tile([128, N2], bf16)             # x2 transposed (d on partitions)
        kstack = persist.tile([128, NG * 128], f32r)    # kT chunk c: [32*(c%4):+32, 128*(c//4):+128]
        vstack = persist.tile([128, NCH * D], bf16)     # v chunk c: [:, 32*c:+32]
        vtstack = persist.tile([128, NG * 128], bf16)   # vT banded like kstack
        qt_rep = persist.tile([128, MSH], f32r)         # qT replicated in 4 bands
        x1t_all = persist.tile([128, MSH], f32r)        # input1 shard transposed
        fusedT = persist.tile([D, MSH], f32)            # v.T @ P.T after band reduce
        na_all = persist.tile([D + 1, 512], f32r)       # augmented LN lhsT; row D = ones

        with (
            tc.tile_pool(name="sc_ps", bufs=2, space="PSUM") as sc_ps,
            tc.tile_pool(name="kv_ps", bufs=1, space="PSUM") as kv_ps,
            tc.tile_pool(name="av_ps", bufs=1, space="PSUM") as av_ps,
            tc.tile_pool(name="pp_ps", bufs=1, space="PSUM") as pp_ps,
            tc.tile_pool(name="x1load", bufs=1) as x1load,
            tc.tile_pool(name="pt", bufs=6) as ptp,
            tc.tile_pool(name="tmp32", bufs=2) as tmp32p,
            tc.tile_pool(name="fb", bufs=2) as fbp,
            tc.tile_pool(name="cent", bufs=2) as centp,
            tc.tile_pool(name="stat", bufs=2) as statp,
            tc.tile_pool(name="lnagg", bufs=8) as lnagg,
            tc.tile_pool(name="outsb", bufs=2) as outsbp,
        ):
            nc.vector.tensor_scalar(
                na_all[D:D + 1, :], x2t[0:1, 0:512], 0.0, 1.0,
                op0=OP.mult, op1=OP.add,
            )

            # ---- input DMAs: the q-prep chain (ident then x1 half 0)
            # gates the stream start, so those go down the queues first;
            # x1 half 0 is split across the SP and Activation DGE queues
            # (the 512B-descriptor row gather runs at ~80GB/s per queue).
            # x1 half 1 only feeds pass 1 (~60us in), so it rides behind
            # the XBARs.
            nc.sync.dma_start(out=identt, in_=id_d[:])
            x1_sbs = []
            for h in range(2):
                x1h = x1load.tile([128, 2, 2, 128], f32, name=f"x1h{h}")
                x1_sbs.append(x1h)
            for q in range(2):
                eng = nc.sync if q == 0 else nc.scalar
                eng.dma_start(
                    out=x1_sbs[0][:, q, :, :],
                    in_=x1[q * 256:(q + 1) * 256, :].rearrange(
                        "(t p) d -> p t d", p=128
                    ),
                )
            nc.sync.dma_start(out=cst, in_=cst_d[:])
            nc.sync.dma_start(out=wkv, in_=wkv_d[:])
            for g in range(4):
                nc.sync.dma_start(
                    out=x2t[:, g * 512:(g + 1) * 512],
                    in_=x2b[g * 512:(g + 1) * 512, :],
                    transpose=True,
                )
            for B in range(1, NB):
                nc.sync.dma_start(
                    out=x2t[:, B * 2048:(B + 1) * 2048],
                    in_=x2b[B * 2048:(B + 1) * 2048, :],
                    transpose=True,
                )
            for q in range(2):
                eng = nc.sync if q == 0 else nc.scalar
                eng.dma_start(
                    out=x1_sbs[1][:, q, :, :],
                    in_=x1[512 + q * 256:512 + (q + 1) * 256, :].rearrange(
                        "(t p) d -> p t d", p=128
                    ),
                )

            # ---- q prep: per-half chains. Pass 0 only needs qT cols
            # 0:512, so half 0 gates the stream (high priority, emitted
            # now); half 1's x1 lands behind the XBARs, so its chain is
            # emitted mid-pass-0 to keep it out of the engine queues' way.
            def q_half(h, tsp, qsp):
                for t in range(4):
                    nc.tensor.transpose(
                        tsp[:, h * 512 + t * 128:h * 512 + (t + 1) * 128],
                        x1_sbs[h][:, t // 2, t % 2, :], ident,
                    )
                nc.vector.tensor_copy(
                    x1t_all[:, h * 512:(h + 1) * 512],
                    tsp[:, h * 512:(h + 1) * 512],
                )
                nc.tensor.matmul(
                    qsp[:, h * 512:(h + 1) * 512],
                    lhsT=wq4_r,
                    rhs=x1t_all[:, h * 512:(h + 1) * 512],
                    start=True,
                    stop=True,
                )
                nc.vector.tensor_scalar_add(
                    qt_rep[:, h * 512:(h + 1) * 512],
                    qsp[:, h * 512:(h + 1) * 512],
                    bq4,
                )

            with tc.high_priority():
                tsp = sc_ps.tile([128, 1024], f32, tag="sc")
                qsp = sc_ps.tile([128, 1024], f32, tag="sc")
                q_half(0, tsp, qsp)

            # ---- k/v prep: 4 batches of 4 groups, all emitted up front ----
            def prep_batch(B):
                pp = kv_ps.tile([128, 1024], f32, tag="kv")
                for gi in range(4):
                    g = 4 * B + gi
                    for j in range(4):
                        nc.tensor.matmul(
                            pp[32 * j:32 * (j + 1), gi * 128:(gi + 1) * 128],
                            lhsT=wk,
                            rhs=x2t[:, (4 * g + j) * 128:(4 * g + j + 1) * 128],
                            start=True,
                            stop=True,
                            tile_position=(0, 32 * j),
                        )
                    nc.vector.tensor_scalar_add(
                        kstack[:, g * 128:(g + 1) * 128],
                        pp[:, gi * 128:(gi + 1) * 128], bk4
                    )
                if B < 2:
                    for ci in range(16):
                        c = 16 * B + ci
                        nc.tensor.matmul(
                            pp[:, 512 + 32 * ci:512 + 32 * (ci + 1)],
                            lhsT=x2t[:, c * 128:(c + 1) * 128],
                            rhs=wv,
                            start=True,
                            stop=True,
                        )
                    nc.vector.tensor_add(
                        vstack[:, B * 512:(B + 1) * 512], pp[:, 512:1024],
                        bv16,
                    )
                else:
                    for gi in range(4):
                        g = 4 * B + gi
                        for j in range(4):
                            nc.tensor.matmul(
                                pp[32 * j:32 * (j + 1),
                                   512 + gi * 128:512 + (gi + 1) * 128],
                                lhsT=wv,
                                rhs=x2t[:, (4 * g + j) * 128:
                                        (4 * g + j + 1) * 128],
                                start=True,
                                stop=True,
                                tile_position=(0, 32 * j),
                            )
                    nc.vector.tensor_scalar_add(
                        vtstack[:, B * 512:(B + 1) * 512], pp[:, 512:1024],
                        bv4,
                    )
                    vs3 = vstack[:].rearrange("p (t d) -> p t d", d=128)
                    for j in range(4):
                        nc.sync.dma_start(
                            out=vs3[:, 4 * B:4 * (B + 1), 32 * j:32 * (j + 1)],
                            in_=vtstack[32 * j:32 * (j + 1),
                                        B * 512:(B + 1) * 512],
                            transpose=True,
                        )

            for B in range(2):
                prep_batch(B)

            # ---- attention stream pieces ----
            def score_exp(p, g):
                m0 = p * 512
                ta = sc_ps.tile([128, 1024], f32, tag="sc")
                tb = sc_ps.tile([128, 1024], f32, tag="sc")
                for j in range(4):
                    t_ = ta if j < 2 else tb
                    c0 = 512 * (j % 2)
                    nc.tensor.matmul(
                        t_[:, c0:c0 + 512],
                        lhsT=kstack[32 * j:32 * (j + 1), g * 128:(g + 1) * 128],
                        rhs=qt_rep[32 * j:32 * (j + 1), m0:m0 + 512],
                        start=True,
                        stop=True,
                        tile_position=(32 * j, 0),
                    )
                pta = ptp.tile([128, 1024], bf16, tag="pt")
                nc.scalar.activation(pta, ta, AF.Exp)
                ptb = ptp.tile([128, 1024], bf16, tag="pt")
                nc.scalar.activation(ptb, tb, AF.Exp)
                return pta, ptb

            def av_quad(g, pts, av_acc):
                pta, ptb = pts
                for j in range(4):
                    c = 4 * g + j
                    pt_ = pta if j < 2 else ptb
                    c0 = 512 * (j % 2)
                    nc.tensor.matmul(
                        av_acc[32 * j:32 * (j + 1), :],
                        lhsT=vstack[:, D * c:D * (c + 1)],
                        rhs=pt_[:, c0:c0 + 512],
                        start=(g == 0),
                        stop=(g == NG - 1),
                        tile_position=(0, 32 * j),
                        skip_group_check=True,
                    )

            def band_reduce(p, av_acc):
                m0 = p * 512
                t1 = tmp32p.tile([D, 512], f32, tag="t1")
                nc.vector.tensor_copy(t1, av_acc[0:32, :])
                t2 = tmp32p.tile([D, 512], f32, tag="t2")
                nc.vector.tensor_add(t2, t1, av_acc[32:64, :])
                nc.vector.tensor_add(t1, t2, av_acc[64:96, :])
                nc.vector.tensor_add(
                    fusedT[:, m0:m0 + 512], t1, av_acc[96:128, :]
                )

            def rsqrt_dve(rstd, ve, p):
                """rstd = ve**-0.5 on the DVE (bit-trick seed + 2 Newton)."""
                sh = lnagg.tile([128, 4], i32, tag="sh")
                nc.vector.tensor_scalar(
                    sh, ve.bitcast(i32), 1, None, op0=OP.logical_shift_right
                )
                s_i = lnagg.tile([128, 4], i32, tag="si")
                nc.vector.tensor_tensor(
                    s_i, magic[:, 4 * p:4 * (p + 1)], sh, op=OP.subtract
                )
                cur = s_i.bitcast(f32)
                for it in range(1):
                    a = lnagg.tile([128, 4], f32, tag=f"a{it}")
                    nc.vector.tensor_tensor(a, cur, cur, op=OP.mult)
                    nc.vector.tensor_tensor(a, a, ve, op=OP.mult)
                    nc.vector.tensor_scalar(
                        a, a, -0.5, 1.5, op0=OP.mult, op1=OP.add
                    )
                    nxt = rstd if it == 0 else lnagg.tile([128, 4], f32, tag="s1")
                    nc.vector.tensor_tensor(nxt, cur, a, op=OP.mult)
                    cur = nxt

            # ---- LayerNorm + output projection for one 512-row pass ----
            def ln_pass(p, psp, ptag):
                m0 = p * 512
                fp1 = psp.tile([128, 512], f32, tag=ptag)
                for bi in range(4):
                    nc.tensor.transpose(
                        fp1[:, 32 * bi:32 * (bi + 1)],
                        fusedT[:, m0 + bi * 128:m0 + (bi + 1) * 128],
                        ident[0:D, 0:D],
                    )
                mv = lnagg.tile([128, 4, 2], f32, tag="mv")
                for bi in range(4):
                    st = statp.tile([128, 6], f32, tag="st")
                    nc.vector.bn_stats(out=st, in_=fp1[:, 32 * bi:32 * (bi + 1)])
                    nc.vector.bn_aggr(out=mv[:, bi, :], in_=st)
                ve = lnagg.tile([128, 4], f32, tag="ve")
                nc.vector.tensor_scalar_add(ve, mv[:, :, 1], epsc)
                rstd = lnagg.tile([128, 4], f32, tag="rs")
                rsqrt_dve(rstd, ve, p)
                cent = centp.tile([128, 128], f32, tag="c")
                for bi in range(4):
                    nc.vector.tensor_scalar(
                        cent[:, 32 * bi:32 * (bi + 1)],
                        fp1[:, 32 * bi:32 * (bi + 1)],
                        mv[:, bi, 0:1], rstd[:, bi:bi + 1],
                        op0=OP.subtract, op1=OP.mult,
                    )
                fp2 = psp.tile([128, 512], f32, tag=ptag)
                for bi in range(4):
                    nc.tensor.transpose(
                        fp2[0:D, 128 * bi:128 * (bi + 1)],
                        cent[:, 32 * bi:32 * (bi + 1)],
                        ident,
                    )
                nc.vector.tensor_copy(na_all[0:D, :], fp2[0:D, 0:512])
                fp3 = psp.tile([128, 512], f32, tag=ptag)
                for bi in range(4):
                    nc.tensor.matmul(
                        fp3[:, 128 * bi:128 * (bi + 1)],
                        lhsT=na_all[:, 128 * bi:128 * (bi + 1)],
                        rhs=woa_r,
                        start=True,
                        stop=True,
                    )
                osb = outsbp.tile([128, 4, DOUT], f32, tag="o")
                nc.vector.tensor_copy(osb, fp3[:].rearrange("p (t d) -> p t d", t=4))
                nc.sync.dma_start(
                    out=out_d[m0:m0 + 512, :].rearrange("(t p) d -> p t d", p=128),
                    in_=osb,
                )

            # ---- main: two sequential m-passes, software-pipelined AV ----
            av_accs = [None, None]
            for p in range(NP):
                pool = av_ps if p == 0 else kv_ps
                av_acc = pool.tile(
                    [128, 512], f32, tag="av" if p == 0 else "kv", name=f"av{p}"
                )
                av_accs[p] = av_acc
                pipe = []
                for g in range(NG):
                    pipe.append(score_exp(p, g))
                    if len(pipe) > 1:
                        av_quad(g - 1, pipe.pop(0), av_acc)
                    if p == 0 and g == 2:
                        prep_batch(2)
                    if p == 0 and g == 6:
                        prep_batch(3)
                    if p == 0 and g == 10:
                        tsp2 = sc_ps.tile([128, 1024], f32, tag="sc")
                        qsp2 = sc_ps.tile([128, 1024], f32, tag="sc")
                        q_half(1, tsp2, qsp2)
                    if p == 1 and g == 6:
                        ln_pass(0, pp_ps, "pp")
                for i, pts in enumerate(pipe):
                    av_quad(NG - len(pipe) + i, pts, av_acc)
                band_reduce(p, av_acc)
            ln_pass(1, sc_ps, "sc")

    nc.compile()
    _CACHE["nc"] = nc
    return nc


def host_inputs(input1, input2, Wq, bq, Wk, bk, Wv, bv, gamma, beta, Wo, bo):
    """Per-core input maps (host-side weight folding + bf16 casts)."""
    import ml_dtypes
    f32 = np.float32
    bf16 = ml_dtypes.bfloat16
    input1 = np.ascontiguousarray(np.asarray(input1, f32))
    x2b = np.ascontiguousarray(np.asarray(input2, f32).astype(bf16))
    woa = np.concatenate(
        [np.asarray(gamma, f32)[:, None] * np.asarray(Wo, f32),
         (np.asarray(beta, f32) @ np.asarray(Wo, f32) + np.asarray(bo, f32))[None, :]],
        axis=0,
    ).astype(f32)
    cst = np.zeros((128, CW), f32)
    cst[:, C_ID:C_ID + 128] = np.eye(128, dtype=f32)
    cst[:, C_BV] = np.tile(np.asarray(bv, f32), 4)
    cst[:, C_BV16:C_BV16 + 512] = np.tile(np.asarray(bv, f32), 16)
    cst[:, C_BQ] = np.tile(np.asarray(bq, f32), 4)
    cst[:, C_BK] = np.tile(np.asarray(bk, f32), 4)
    cst[:, C_EPS] = LN_EPS
    cst[0:D + 1, C_WOA:C_WOA + 128] = woa
    cst[:, C_WQ:C_WQ + 128] = np.tile(np.asarray(Wq, f32), (1, 4))
    wkv = np.concatenate(
        [np.asarray(Wk, f32), np.asarray(Wv, f32)], axis=1
    ).astype(bf16)
    common = {"x2b": x2b, "cst": cst, "wkv": np.ascontiguousarray(wkv),
              "identd": np.eye(128, dtype=f32)}
    return [
        dict(common, x1=input1[c * MSH:(c + 1) * MSH]) for c in range(NCORES)
    ]


def kernel(input1, input2, Wq, bq, Wk, bk, Wv, bv, gamma, beta, Wo, bo):
    _import_concourse()
    from concourse.bass_utils import run_bass_kernel_spmd

    nc = build()
    in_maps = host_inputs(
        input1, input2, Wq, bq, Wk, bk, Wv, bv, gamma, beta, Wo, bo
    )
    res = run_bass_kernel_spmd(nc, in_maps, list(range(NCORES)))
    return np.concatenate(
        [np.asarray(res.results[c]["out"]) for c in range(NCORES)], axis=0
    ).astype(np.float32)


# revision 21
# speedup vs baseline: 1.0059x; 1.0059x over previous
"""Trainium2 Bass kernel for fused attention + LayerNorm + projection.

Computation (per reference):
    q = input1 @ Wq + bq                       [8192, 32]
    k = input2 @ Wk + bk                       [8192, 32]
    v = input2 @ Wv + bv                       [8192, 32]
    P = softmax(q @ k.T, axis=-1)              [8192, 8192]
    fused = P @ v                              [8192, 32]
    out = LayerNorm(fused) * gamma + beta @ Wo + bo   [8192, 128]

Sharding: data-parallel over rows of input1 (1024 rows per core, 8 cores);
input2 and weights replicated.

Algebraic simplifications (validated vs reference, rel err ~6e-3):
  - softmax normalization (and max-subtraction) skipped: LayerNorm is
    invariant to a positive per-row scale, so exp(s) @ v is normalized for
    free by LN.
  - gamma folded into Wo (diag(gamma) @ Wo), beta/bo folded into an extra
    contraction row via an augmented ones-row, on the host.

Dataflow per core (ACT-exp is the roofline: 8.4M exps at 1 elem/lane/cyc
@1.2GHz ~= 64us; the PE runs everything as f32r/bf16 at ~1.2GHz under the
chip's 50%-utilization clock cap, so PE and ACT are nearly balanced and
everything is arranged to overlap):
  - x2 is cast to bf16 on host and DMA'd HBM->SBUF through the XBAR
    transpose unit: x2T [128d, 8192n] lands directly with the contraction
    dim on partitions - zero PE transposes for the k/v projections. The
    first 4 groups ride individual 512-row XBARs (lower latency), the
    rest 2048-row ones (lower queue cost).
  - prep is batched 4 groups (16 chunks) per dedicated 2-bank PSUM tile:
    4-way column-tiled k quads into bank0 (kT banded into kstack f32r,
    chunk c at partitions 32*(c%4)). v: batches 0-1 (consumed early) use
    direct per-chunk matmuls into natural [n,32] layout; batches 2-3 use
    4-way column-tiled vT quads + XBAR SBUF->SBUF band transposes into
    vstack (less PE time, more latency). Batches 0-1 are emitted up
    front, 2-3 mid-pass-0, so prep fills the stream's PE slack.
  - q: x1 stays f32 for precision; half 0 is split across both DGE
    queues, PE-transposed, then ONE f32r matmul against Wq tiled 4x in
    columns writes qT replicated into all 4 partition bands of qt_rep
    (a [32,N] f32r matmul dst fails the ISA; plain fp32 matmuls split
    into HI/LO pairs and run 4x slower). Half 1 feeds only pass 1 and is
    deferred behind the x2 XBARs.
  - scores: per (pass, group) ONE 4-way row-tiled f32r matmul quad
    (tile_position=(32j,0), concurrent on the PE sub-arrays) writes 4
    PSUM banks = scoresT for 4 chunks x 512 m.
  - exp on ACT straight out of PSUM, 2x [128,1024] bf16 -> pt. Emission
    is software-pipelined (scores g | exp g | AV g-1) so the PE finishes
    each score quad inside the previous exp's shadow and ACT never gaps.
  - AV: 4-way col-tiled bf16 quad accumulates fusedT into one PSUM bank
    across all 16 groups (start on g==0, stop on g==15). Pass 1's
    accumulator reuses the prep pool's bank so it never waits on pass
    0's band reduce.
  - m is processed in 2 sequential passes of 512 rows; pass 0's LayerNorm
    + output projection overlaps pass 1's attention stream (emitted at
    pass-1 group 6 so it never head-of-line-blocks the PE queue).
  - PSUM: 2x [128,1024] score tiles (4 banks) + prep tile (2 banks) +
    1 AV bank + 1 LN bank = 8.
  - LayerNorm (batched, 4 blocks per pass): bn_stats/bn_aggr straight
    from the transpose PSUM; rstd is computed on the DVE with the
    bit-trick rsqrt + one Newton step so the ACT engine NEVER leaves the
    exp table set (a Ln/Sqrt activation would cost two ~2.7us
    ACT_TABLE_LOAD+DRAIN stalls per pass); projection through an
    augmented [33,512] lhsT with a persistent ones row; one rearranged
    output DMA per pass.
"""

import os
import sys

import numpy as np

N1 = 8192
N2 = 8192
DIN = 128
D = 32
DOUT = 128
NCORES = 8
MSH = N1 // NCORES          # rows per core
NCH = N2 // 128             # 64 in2 chunks
NG = NCH // 4               # 16 groups of 4 chunks
NB = NG // 4                # 4 prep batches of 4 groups
NP = MSH // 512             # 2 m-passes of 512 rows
LN_EPS = 1e-5

_CACHE = {}


def _import_concourse():
    try:
        import concourse.bass  # noqa: F401
    except ImportError:
        for p in ("/opt/trn_rl_repo", os.path.expanduser("~/.axon_site/_ro/trn_rl_repo")):
            if os.path.isdir(p) and p not in sys.path:
                sys.path.insert(0, p)


# Packed f32 consts layout (columns of the [128, CW] "cst" tensor).
C_ID = 0          # ident [128,128]
C_BV = 128        # bv4   [128,1]
C_BQ = 129        # bq4   [128,1]
C_BK = 130        # bk4   [128,1]
C_EPS = 131       # epsc  [128,1]
C_WOA = 132       # woa   [33,128] (rows 0:33)
C_WQ = 260        # wq4   [128,128] (Wq tiled 4x in cols)
C_BV16 = 388      # bv16  [128,512]
CW = 900


def build():
    """Build (and cache) the compiled single-core SPMD Bass program."""
    if "nc" in _CACHE:
        return _CACHE["nc"]
    _import_concourse()
    import concourse.bacc as bacc
    import concourse.tile as tile
    from concourse import mybir

    f32 = mybir.dt.float32
    f32r = mybir.dt.float32r
    i32 = mybir.dt.int32
    bf16 = mybir.dt.bfloat16
    AF = mybir.ActivationFunctionType
    OP = mybir.AluOpType

    nc = bacc.Bacc(None, target_bir_lowering=False, debug=False)

    x1 = nc.dram_tensor("x1", [MSH, DIN], f32, kind="ExternalInput")
    x2b = nc.dram_tensor("x2b", [N2, DIN], bf16, kind="ExternalInput")
    cst_d = nc.dram_tensor("cst", [128, CW], f32, kind="ExternalInput")
    id_d = nc.dram_tensor("identd", [128, 128], f32, kind="ExternalInput")
    wkv_d = nc.dram_tensor("wkv", [DIN, 2 * D], bf16, kind="ExternalInput")
    out_d = nc.dram_tensor("out", [MSH, DOUT], f32, kind="ExternalOutput")

    from contextlib import ExitStack

    with tile.TileContext(nc) as tc, ExitStack() as outer:
        consts = outer.enter_context(tc.tile_pool(name="consts", bufs=1))
        persist = outer.enter_context(tc.tile_pool(name="persist", bufs=1))

        cst = consts.tile([128, CW], f32)
        wkv = consts.tile([DIN, 2 * D], bf16)
        identt = consts.tile([128, 128], f32)
        ident = identt[:]
        bv4 = cst[:, C_BV:C_BV + 1]
        bv16 = cst[:, C_BV16:C_BV16 + 512]
        bq4 = cst[:, C_BQ:C_BQ + 1]
        bk4 = cst[:, C_BK:C_BK + 1]
        epsc = cst[:, C_EPS:C_EPS + 1]
        wk = wkv[:, 0:D]
        wv = wkv[:, D:2 * D]

        woa_r = consts.tile([D + 1, DOUT], f32r)
        nc.vector.tensor_copy(woa_r, cst[0:D + 1, C_WOA:C_WOA + 128])
        wq4_r = consts.tile([DIN, 128], f32r)
        nc.vector.tensor_copy(wq4_r, cst[:, C_WQ:C_WQ + 128])
        magic = consts.tile([128, NP * 4], i32)
        nc.vector.memset(magic, 0x5F3759DF)

        # Pull the exp table load (~2.7us) into the startup window; source
        # from a memset tile so it gates on no DMA (the ACT DGE queue
        # carries the second x1 half right behind it).
        warm = consts.tile([1, 8], f32)
        nc.vector.memset(warm, 0.0)
        nc.scalar.activation(warm, warm, AF.Exp)

        x2t = persist.tile([128, N2], bf16)             # x2 transposed (d on partitions)
        kstack = persist.tile([128, NG * 128], f32r)    # kT chunk c: [32*(c%4):+32, 128*(c//4):+128]
        vstack = persist.tile([128, NCH * D], bf16)     # v chunk c: [:, 32*c:+32]
        vtstack = persist.tile([128, NG * 128], bf16)   # vT banded like kstack
        qt_rep = persist.tile([128, MSH], f32r)         # qT replicated in 4 bands
        x1t_all = persist.tile([128, MSH], f32r)        # input1 shard transposed
        fusedT = persist.tile([D, MSH], f32)            # v.T @ P.T after band reduce
        na_all = persist.tile([D + 1, 512], f32r)       # augmented LN lhsT; row D = ones

        with (
            tc.tile_pool(name="sc_ps", bufs=2, space="PSUM") as sc_ps,
            tc.tile_pool(name="kv_ps", bufs=1, space="PSUM") as kv_ps,
            tc.tile_pool(name="av_ps", bufs=1, space="PSUM") as av_ps,
            tc.tile_pool(name="pp_ps", bufs=1, space="PSUM") as pp_ps,
            tc.tile_pool(name="x1load", bufs=1) as x1load,
            tc.tile_pool(name="pt", bufs=6) as ptp,
            tc.tile_pool(name="tmp32", bufs=2) as tmp32p,
            tc.tile_pool(name="fb", bufs=2) as fbp,
            tc.tile_pool(name="cent", bufs=2) as centp,
            tc.tile_pool(name="stat", bufs=2) as statp,
            tc.tile_pool(name="lnagg", bufs=8) as lnagg,
            tc.tile_pool(name="outsb", bufs=2) as outsbp,
        ):
            nc.vector.tensor_scalar(
                na_all[D:D + 1, :], x2t[0:1, 0:512], 0.0, 1.0,
                op0=OP.mult, op1=OP.add,
            )

            # ---- input DMAs: the q-prep chain (ident then x1 half 0)
            # gates the stream start, so those go down the queues first;
            # x1 half 0 is split across the SP and Activation DGE queues
            # (the 512B-descriptor row gather runs at ~80GB/s per queue).
            # x1 half 1 only feeds pass 1 (~60us in), so it rides behind
            # the XBARs.
            nc.sync.dma_start(out=identt, in_=id_d[:])
            x1_sbs = []
            for h in range(2):
                x1h = x1load.tile([128, 2, 2, 128], f32, name=f"x1h{h}")
                x1_sbs.append(x1h)
            for q in range(2):
                eng = nc.sync if q == 0 else nc.scalar
                eng.dma_start(
                    out=x1_sbs[0][:, q, :, :],
                    in_=x1[q * 256:(q + 1) * 256, :].rearrange(
                        "(t p) d -> p t d", p=128
                    ),
                )
            nc.sync.dma_start(out=cst, in_=cst_d[:])
            nc.sync.dma_start(out=wkv, in_=wkv_d[:])
            for g in range(4):
                nc.sync.dma_start(
                    out=x2t[:, g * 512:(g + 1) * 512],
                    in_=x2b[g * 512:(g + 1) * 512, :],
                    transpose=True,
                )
            for B in range(1, NB):
                nc.sync.dma_start(
                    out=x2t[:, B * 2048:(B + 1) * 2048],
                    in_=x2b[B * 2048:(B + 1) * 2048, :],
                    transpose=True,
                )
            for q in range(2):
                eng = nc.sync if q == 0 else nc.scalar
                eng.dma_start(
                    out=x1_sbs[1][:, q, :, :],
                    in_=x1[512 + q * 256:512 + (q + 1) * 256, :].rearrange(
                        "(t p) d -> p t d", p=128
                    ),
                )

            # ---- q prep: per-half chains. Pass 0 only needs qT cols
            # 0:512, so half 0 gates the stream (high priority, emitted
            # now); half 1's x1 lands behind the XBARs, so its chain is
            # emitted mid-pass-0 to keep it out of the engine queues' way.
            def q_half(h, tsp, qsp):
                for t in range(4):
                    nc.tensor.transpose(
                        tsp[:, h * 512 + t * 128:h * 512 + (t + 1) * 128],
                        x1_sbs[h][:, t // 2, t % 2, :], ident,
                    )
                nc.vector.tensor_copy(
                    x1t_all[:, h * 512:(h + 1) * 512],
                    tsp[:, h * 512:(h + 1) * 512],
                )
                nc.tensor.matmul(
                    qsp[:, h * 512:(h + 1) * 512],
                    lhsT=wq4_r,
                    rhs=x1t_all[:, h * 512:(h + 1) * 512],
                    start=True,
                    stop=True,
                )
                nc.vector.tensor_scalar_add(
                    qt_rep[:, h * 512:(h + 1) * 512],
                    qsp[:, h * 512:(h + 1) * 512],
                    bq4,
                )

            with tc.high_priority():
                tsp = sc_ps.tile([128, 1024], f32, tag="sc")
                qsp = sc_ps.tile([128, 1024], f32, tag="sc")
                q_half(0, tsp, qsp)

            # ---- k/v prep: 4 batches of 4 groups, all emitted up front ----
            def prep_batch(B):
                pp = kv_ps.tile([128, 1024], f32, tag="kv")
                for gi in range(4):
                    g = 4 * B + gi
                    for j in range(4):
                        nc.tensor.matmul(
                            pp[32 * j:32 * (j + 1), gi * 128:(gi + 1) * 128],
                            lhsT=wk,
                            rhs=x2t[:, (4 * g + j) * 128:(4 * g + j + 1) * 128],
                            start=True,
                            stop=True,
                            tile_position=(0, 32 * j),
                        )
                    nc.vector.tensor_scalar_add(
                        kstack[:, g * 128:(g + 1) * 128],
                        pp[:, gi * 128:(gi + 1) * 128], bk4
                    )
                if B < 2:
                    for ci in range(16):
                        c = 16 * B + ci
                        nc.tensor.matmul(
                            pp[:, 512 + 32 * ci:512 + 32 * (ci + 1)],
                            lhsT=x2t[:, c * 128:(c + 1) * 128],
                            rhs=wv,
                            start=True,
                            stop=True,
                        )
                    nc.vector.tensor_add(
                        vstack[:, B * 512:(B + 1) * 512], pp[:, 512:1024],
                        bv16,
                    )
                else:
                    for gi in range(4):
                        g = 4 * B + gi
                        for j in range(4):
                            nc.tensor.matmul(
                                pp[32 * j:32 * (j + 1),
                                   512 + gi * 128:512 + (gi + 1) * 128],
                                lhsT=wv,
                                rhs=x2t[:, (4 * g + j) * 128:
                                        (4 * g + j + 1) * 128],
                                start=True,
                                stop=True,
                                tile_position=(0, 32 * j),
                            )
                    nc.vector.tensor_scalar_add(
                        vtstack[:, B * 512:(B + 1) * 512], pp[:, 512:1024],
                        bv4,
                    )
                    vs3 = vstack[:].rearrange("p (t d) -> p t d", d=128)
                    for j in range(4):
                        nc.sync.dma_start(
                            out=vs3[:, 4 * B:4 * (B + 1), 32 * j:32 * (j + 1)],
                            in_=vtstack[32 * j:32 * (j + 1),
                                        B * 512:(B + 1) * 512],
                            transpose=True,
                        )

            for B in range(2):
                prep_batch(B)

            # ---- attention stream pieces ----
            def score_exp(p, g):
                m0 = p * 512
                ta = sc_ps.tile([128, 1024], f32, tag="sc")
                tb = sc_ps.tile([128, 1024], f32, tag="sc")
                for j in range(4):
                    t_ = ta if j < 2 else tb
                    c0 = 512 * (j % 2)
                    nc.tensor.matmul(
                        t_[:, c0:c0 + 512],
                        lhsT=kstack[32 * j:32 * (j + 1), g * 128:(g + 1) * 128],
                        rhs=qt_rep[32 * j:32 * (j + 1), m0:m0 + 512],
                        start=True,
                        stop=True,
                        tile_position=(32 * j, 0),
                    )
                pta = ptp.tile([128, 1024], bf16, tag="pt")
                nc.scalar.activation(pta, ta, AF.Exp)
                ptb = ptp.tile([128, 1024], bf16, tag="pt")
                nc.scalar.activation(ptb, tb, AF.Exp)
                return pta, ptb

            def av_quad(g, pts, av_acc):
                pta, ptb = pts
                for j in range(4):
                    c = 4 * g + j
                    pt_ = pta if j < 2 else ptb
                    c0 = 512 * (j % 2)
                    nc.tensor.matmul(
                        av_acc[32 * j:32 * (j + 1), :],
                        lhsT=vstack[:, D * c:D * (c + 1)],
                        rhs=pt_[:, c0:c0 + 512],
                        start=(g == 0),
                        stop=(g == NG - 1),
                        tile_position=(0, 32 * j),
                        skip_group_check=True,
                    )

            def band_reduce(p, av_acc):
                m0 = p * 512
                t1 = tmp32p.tile([D, 512], f32, tag="t1")
                nc.vector.tensor_copy(t1, av_acc[0:32, :])
                t2 = tmp32p.tile([D, 512], f32, tag="t2")
                nc.vector.tensor_add(t2, t1, av_acc[32:64, :])
                nc.vector.tensor_add(t1, t2, av_acc[64:96, :])
                nc.vector.tensor_add(
                    fusedT[:, m0:m0 + 512], t1, av_acc[96:128, :]
                )

            def rsqrt_dve(rstd, ve, p):
                """rstd = ve**-0.5 on the DVE (bit-trick seed + 2 Newton)."""
                sh = lnagg.tile([128, 4], i32, tag="sh")
                nc.vector.tensor_scalar(
                    sh, ve.bitcast(i32), 1, None, op0=OP.logical_shift_right
                )
                s_i = lnagg.tile([128, 4], i32, tag="si")
                nc.vector.tensor_tensor(
                    s_i, magic[:, 4 * p:4 * (p + 1)], sh, op=OP.subtract
                )
                cur = s_i.bitcast(f32)
                for it in range(1):
                    a = lnagg.tile([128, 4], f32, tag=f"a{it}")
                    nc.vector.tensor_tensor(a, cur, cur, op=OP.mult)
                    nc.vector.tensor_tensor(a, a, ve, op=OP.mult)
                    nc.vector.tensor_scalar(
                        a, a, -0.5, 1.5, op0=OP.mult, op1=OP.add
                    )
                    nxt = rstd if it == 0 else lnagg.tile([128, 4], f32, tag="s1")
                    nc.vector.tensor_tensor(nxt, cur, a, op=OP.mult)
                    cur = nxt

            # ---- LayerNorm + output projection for one 512-row pass ----
            def ln_pass(p, psp, ptag):
                m0 = p * 512
                fp1 = psp.tile([128, 512], f32, tag=ptag)
                for bi in range(4):
                    nc.tensor.transpose(
                        fp1[:, 32 * bi:32 * (bi + 1)],
                        fusedT[:, m0 + bi * 128:m0 + (bi + 1) * 128],
                        ident[0:D, 0:D],
                    )
                mv = lnagg.tile([128, 4, 2], f32, tag="mv")
                for bi in range(4):
                    st = statp.tile([128, 6], f32, tag="st")
                    nc.vector.bn_stats(out=st, in_=fp1[:, 32 * bi:32 * (bi + 1)])
                    nc.vector.bn_aggr(out=mv[:, bi, :], in_=st)
                ve = lnagg.tile([128, 4], f32, tag="ve")
                nc.vector.tensor_scalar_add(ve, mv[:, :, 1], epsc)
                rstd = lnagg.tile([128, 4], f32, tag="rs")
                rsqrt_dve(rstd, ve, p)
                cent = centp.tile([128, 128], f32, tag="c")
                for bi in range(4):
                    nc.vector.tensor_scalar(
                        cent[:, 32 * bi:32 * (bi + 1)],
                        fp1[:, 32 * bi:32 * (bi + 1)],
                        mv[:, bi, 0:1], rstd[:, bi:bi + 1],
                        op0=OP.subtract, op1=OP.mult,
                    )
                fp2 = psp.tile([128, 512], f32, tag=ptag)
                for bi in range(4):
                    nc.tensor.transpose(
                        fp2[0:D, 128 * bi:128 * (bi + 1)],
                        cent[:, 32 * bi:32 * (bi + 1)],
                        ident,
                    )
                nc.vector.tensor_copy(na_all[0:D, :], fp2[0:D, 0:512])
                fp3 = psp.tile([128, 512], f32, tag=ptag)
                for bi in range(4):
                    nc.tensor.matmul(
                        fp3[:, 128 * bi:128 * (bi + 1)],
                        lhsT=na_all[:, 128 * bi:128 * (bi + 1)],
                        rhs=woa_r,
                        start=True,
                        stop=True,
                    )
                osb = outsbp.tile([128, 4, DOUT], f32, tag="o")
                nc.vector.tensor_copy(osb, fp3[:].rearrange("p (t d) -> p t d", t=4))
                nc.sync.dma_start(
                    out=out_d[m0:m0 + 512, :].rearrange("(t p) d -> p t d", p=128),
                    in_=osb,
                )

            # ---- main: two sequential m-passes, software-pipelined AV ----
            av_accs = [None, None]
            for p in range(NP):
                pool = av_ps if p == 0 else kv_ps
                av_acc = pool.tile(
                    [128, 512], f32, tag="av" if p == 0 else "kv", name=f"av{p}"
                )
                av_accs[p] = av_acc
                pipe = []
                for g in range(NG):
                    pipe.append(score_exp(p, g))
                    if len(pipe) > 1:
                        av_quad(g - 1, pipe.pop(0), av_acc)
                    if p == 0 and g == 2:
                        prep_batch(2)
                    if p == 0 and g == 6:
                        prep_batch(3)
                    if p == 0 and g == 10:
                        tsp2 = sc_ps.tile([128, 1024], f32, tag="sc")
                        qsp2 = sc_ps.tile([128, 1024], f32, tag="sc")
                        q_half(1, tsp2, qsp2)
                    if p == 1 and g == 6:
                        ln_pass(0, pp_ps, "pp")
                for i, pts in enumerate(pipe):
                    av_quad(NG - len(pipe) + i, pts, av_acc)
                band_reduce(p, av_acc)
            ln_pass(1, sc_ps, "sc")

    nc.compile()
    _CACHE["nc"] = nc
    return nc


def host_inputs(input1, input2, Wq, bq, Wk, bk, Wv, bv, gamma, beta, Wo, bo):
    """Per-core input maps (host-side weight folding + bf16 casts)."""
    import ml_dtypes
    f32 = np.float32
    bf16 = ml_dtypes.bfloat16
    input1 = np.ascontiguousarray(np.asarray(input1, f32))
    x2b = np.ascontiguousarray(np.asarray(input2, f32).astype(bf16))
    woa = np.concatenate(
        [np.asarray(gamma, f32)[:, None] * np.asarray(Wo, f32),
         (np.asarray(beta, f32) @ np.asarray(Wo, f32) + np.asarray(bo, f32))[None, :]],
        axis=0,
    ).astype(f32)
    cst = np.zeros((128, CW), f32)
    cst[:, C_ID:C_ID + 128] = np.eye(128, dtype=f32)
    cst[:, C_BV] = np.tile(np.asarray(bv, f32), 4)
    cst[:, C_BV16:C_BV16 + 512] = np.tile(np.asarray(bv, f32), 16)
    cst[:, C_BQ] = np.tile(np.asarray(bq, f32), 4)
    cst[:, C_BK] = np.tile(np.asarray(bk, f32), 4)
    cst[:, C_EPS] = LN_EPS
    cst[0:D + 1, C_WOA:C_WOA + 128] = woa
    cst[:, C_WQ:C_WQ + 128] = np.tile(np.asarray(Wq, f32), (1, 4))
    wkv = np.concatenate(
        [np.asarray(Wk, f32), np.asarray(Wv, f32)], axis=1
    ).astype(bf16)
    common = {"x2b": x2b, "cst": cst, "wkv": np.ascontiguousarray(wkv),
              "identd": np.eye(128, dtype=f32)}
    return [
        dict(common, x1=input1[c * MSH:(c + 1) * MSH]) for c in range(NCORES)
    ]


def kernel(input1, input2, Wq, bq, Wk, bk, Wv, bv, gamma, beta, Wo, bo):
    _import_concourse()
    from concourse.bass_utils import run_bass_kernel_spmd

    nc = build()
    in_maps = host_inputs(
        input1, input2, Wq, bq, Wk, bk, Wv, bv, gamma, beta, Wo, bo
    )
    res = run_bass_kernel_spmd(nc, in_maps, list(range(NCORES)))
    return np.concatenate(
        [np.asarray(res.results[c]["out"]) for c in range(NCORES)], axis=0
    ).astype(np.float32)
